# revision 14
# baseline (speedup 1.0000x reference)
"""Causal multi-head self-attention (b=4, s=2048, d_model=1024, 16 heads) on 8
Trainium2 NeuronCores.

Sharding: core c handles batch c//2 and head-group c%2 (8 of 16 heads):
  - wqkv row-split by head (tensor parallel), wo column-split by head.
  - Each core returns the partial output projection [s, d_model] for its head
    group; the host sums the two partials of each batch while unsharding (the
    pairwise all-reduce of the TP split).

v2 restructure (from 465us baseline) driven by trace analysis:
  - The baseline lost ~150us to HAM clock-gate oscillation (PE idle gaps of
    2-10us at phase seams / chunk tails re-throttle the PE to 1.2 GHz) and
    ~20us to a descriptor-flooded startup (a [128,16,8,1] ones DMA emitted
    ~16k 2-byte descriptors before the first weight load).
  - All of x (4MB) + all weights now live in SBUF, loaded once (the baseline
    re-streamed x per projection phase: 20MB, starving the PE).
  - The ones column of V comes from one strided memset, not a DMA.
  - Emission order keeps the PE continuously fed: V proj, QK-block(0), then
    attention C(0..3) with the remaining QK projection emitted as ~1.7us
    fillers at every chunk tail (where the softmax-denominator chain used to
    stall the PE), and the output projection D interleaved into C(3)'s tails.
  - The AV PSUM tile is freed early: unnormalized y rows drain to SBUF (DVE)
    and denominators to f32r staging (Scalar) right after the last AV matmul;
    the normalize multiply runs later from SBUF off the critical path.
  - Output partials ship as f16 (host sums in f32).

Per-core dataflow (heavy matmuls bf16, fp32 PSUM; softmax denominator f32r):
  V) v[t, f] token-major, stored [t, ktile, head, 65] with a ones column per
     head -- the AV matmul then yields softmax denominators for free.
  QK) qkT[f, t] feature-major per 128-row f-tile with RoPE fused:
      qk' = cos * qk + DMA-swap-add( sin_pm * qk ), sin_pm = [+sin | -sin],
      SWDGE accum_op=add swaps the 32-row halves of each 64-row head block.
  C) per (head pair, q-chunk of 512) over causal k-tiles of 128:
      scores^T two K=64 matmuls (heads at partition halves); causal mask via
      identity @ mtri accumulated onto the diagonal PSUM block; exp on ScalarE
      (scale=1/8 folded); AV with lhsT = [V_h | 1] -> PSUM [65, q], row 64 =
      denominator; drain PSUM early; ones2 K=2 matmul broadcasts denominators
      across partitions; DVE approx-reciprocal; normalize SBUF->SBUF.
  D) output projection vs wo columns, f16 partial DMA'd out; host sums core
     pairs while unsharding.
"""

import sys

if "/opt/trn_rl_repo" not in sys.path:
    sys.path.insert(0, "/opt/trn_rl_repo")

from contextlib import ExitStack

import numpy as np

import concourse.bass as bass  # noqa: F401
import concourse.tile as tile
from concourse import bacc, mybir
from concourse.bass_utils import run_bass_kernel_spmd

F32 = mybir.dt.float32
F32R = mybir.dt.float32r
F16 = mybir.dt.float16
BF16 = mybir.dt.bfloat16
EXP = mybir.ActivationFunctionType.Exp
MULT = mybir.AluOpType.mult
ADD = mybir.AluOpType.add

# Problem constants
B, S_FULL, D = 4, 2048, 1024
NH_CORE = 8      # heads per core
DH = 64          # head dim
FQK = 1024       # Q+K features per core
FV = 512         # V features per core
P = 128
TCH = 512        # q/t chunk size
NEG = -1.0e30
ROPE_THETA = 10000.0
SCALE = 1.0 / 8.0  # 1/sqrt(DH)

_CACHE = {}


def _emit(nc, tc, S, xT, wqkT, wvT, woT, cosF, sinFpm, mtri, ident, ones2, outp):
    n_tch = S // TCH          # 4
    n_kt = S // P             # 16
    n_hp = NH_CORE // 2       # 4
    mm = nc.tensor.matmul

    with ExitStack() as ctx:
        # ---------- persistent buffers ----------
        persist = ctx.enter_context(tc.tile_pool(name="persist", bufs=1))
        xsb = [
            persist.tile([P, 8, TCH], BF16, tag=f"xsb{t}", name=f"xsb{t}")
            for t in range(n_tch)
        ]
        wqk_sb = persist.tile([P, 8, FQK], BF16, tag="wqk")
        wv_sb = persist.tile([P, 8, FV], BF16, tag="wv")
        wo_sb = persist.tile([P, 4, D], BF16, tag="wo")
        cos_sb = persist.tile([P, S], F32, tag="cos")
        sin_sb = persist.tile([P, S], F32, tag="sin")
        qkT = [
            persist.tile([P, S], BF16, tag=f"qkT{ft}", name=f"qkT{ft}")
            for ft in range(8)
        ]
        vbuf = persist.tile([P, n_kt, NH_CORE, DH + 1], BF16, tag="vbuf")
        yT = [
            persist.tile([P, S], BF16, tag=f"yT{hp}", name=f"yT{hp}")
            for hp in range(n_hp)
        ]
        ident_sb = persist.tile([P, P], BF16, tag="ident")
        mtri_sb = persist.tile([P, P], BF16, tag="mtri")
        ones2_sb = persist.tile([2, P], F32R, tag="ones2")
        scr = persist.tile([1, P], BF16, tag="scr")

        # input DMAs, most-urgent first (wv + x feed the V projection); all
        # host arrays are pre-rearranged partition-major so each load is one
        # contiguous descriptor per partition
        nc.sync.dma_start(wv_sb[:, 0:4, :], wvT.ap()[:, 0:4, :])
        nc.sync.dma_start(wv_sb[:, 4:8, :], wvT.ap()[:, 4:8, :])
        nc.sync.dma_start(xsb[0][:, 0:4, :], xT.ap()[:, 0, 0:4, :])
        nc.sync.dma_start(xsb[0][:, 4:8, :], xT.ap()[:, 0, 4:8, :])
        for t in range(1, n_tch):
            nc.sync.dma_start(xsb[t][:], xT.ap()[:, t, :, :])
        nc.sync.dma_start(wqk_sb[:], wqkT.ap()[:, :, :])
        nc.sync.dma_start(cos_sb[:], cosF.ap()[:, :])
        nc.sync.dma_start(sin_sb[:], sinFpm.ap()[:, :])
        nc.sync.dma_start(ident_sb[:], ident.ap()[:, :])
        nc.sync.dma_start(mtri_sb[:], mtri.ap()[:, :])
        nc.sync.dma_start(ones2_sb[:], ones2.ap()[:, :])
        nc.sync.dma_start(wo_sb[:], woT.ap()[:, :, :])
        # ones column of V (softmax denominator trick) -- memset, not DMA
        nc.vector.memset(vbuf[:, :, :, DH : DH + 1], 1.0)
        # preload the ScalarE exp table before C needs it
        nc.vector.memset(scr[:], 0.0)
        nc.scalar.activation(scr[:], scr[:], EXP, scale=SCALE)

        # ---------- V projection (token-major) ----------
        with ExitStack() as vb:
            v_ps = vb.enter_context(tc.tile_pool(name="v_psum", bufs=2, space="PSUM"))
            for kt in range(n_kt):
                tci, tti = divmod(kt, n_tch)
                vps = v_ps.tile([P, FV], F32, tag="vps", name="vps")
                for ec in range(8):
                    mm(
                        vps[:],
                        xsb[tci][:, ec, tti * P : (tti + 1) * P],
                        wv_sb[:, ec, :],
                        start=(ec == 0),
                        stop=(ec == 7),
                    )
                nc.scalar.copy(vbuf[:, kt, :, 0:DH], vps[:])

        # ---------- C-epoch pools (PSUM: sp 4 banks, avp 2, aux 2) ----------
        apool = ctx.enter_context(tc.tile_pool(name="btmp", bufs=2))
        epool = ctx.enter_context(tc.tile_pool(name="expS", bufs=5))
        d2pool = ctx.enter_context(tc.tile_pool(name="den2", bufs=2))
        upool = ctx.enter_context(tc.tile_pool(name="ustage", bufs=4))
        rpool = ctx.enter_context(tc.tile_pool(name="recb", bufs=2))
        opool = ctx.enter_context(tc.tile_pool(name="outsb", bufs=3))
        s_ps = ctx.enter_context(tc.tile_pool(name="s_psum", bufs=2, space="PSUM"))
        av_ps = ctx.enter_context(tc.tile_pool(name="av_psum", bufs=1, space="PSUM"))
        aux_ps = ctx.enter_context(tc.tile_pool(name="aux_psum", bufs=2, space="PSUM"))

        btbuf = {}

        def qk_subblock(ft, tci):
            # one f-tile x one token chunk of the QK projection, RoPE fused
            tsl = slice(tci * TCH, (tci + 1) * TCH)
            if tci == 0:
                btbuf[ft] = apool.tile([P, S], BF16, tag="bt", name=f"bt{ft}")
            btf = btbuf[ft]
            ps = aux_ps.tile([P, TCH], F32, tag="aux", name="qkps")
            for ec in range(8):
                mm(
                    ps[:],
                    wqk_sb[:, ec, ft * P : (ft + 1) * P],
                    xsb[tci][:, ec, :],
                    start=(ec == 0),
                    stop=(ec == 7),
                )
            nc.vector.tensor_tensor(qkT[ft][:, tsl], ps[:], cos_sb[:, tsl], MULT)
            nc.vector.tensor_tensor(btf[:, tsl], ps[:], sin_sb[:, tsl], MULT)
            if tci == n_tch - 1:
                # rotate-half: swap-add the 32-row halves of each 64-row block
                for blk in range(4):
                    a = blk * 32
                    c2 = a ^ 32
                    nc.gpsimd.dma_start(
                        qkT[ft][c2 : c2 + 32, :], btf[a : a + 32, :], accum_op=ADD
                    )

        def d_group(qci):
            # output projection for the 4 token tiles of chunk qci
            for tti in range(qci * n_tch, qci * n_tch + 4):
                tsl = slice(tti * P, (tti + 1) * P)
                for jc in range(2):
                    jsl = slice(jc * TCH, (jc + 1) * TCH)
                    op = aux_ps.tile([P, TCH], F32, tag="aux", name="dps")
                    for cc in range(4):
                        mm(
                            op[:],
                            yT[cc][:, tsl],
                            wo_sb[:, cc, jsl],
                            start=(cc == 0),
                            stop=(cc == 3),
                        )
                    ot = opool.tile([P, TCH], F16, tag="ot", name="ot")
                    if jc == 0:
                        nc.scalar.copy(ot[:], op[:])
                    else:
                        nc.vector.tensor_copy(ot[:], op[:])
                    nc.sync.dma_start(outp.ap()[tsl, jsl], ot[:])

        def c_chunk(hp, qci, fillers, pending):
            """Emit one (head pair, q-chunk). The first two k-tiles' scores +
            exp are emitted BEFORE the previous chunk's tail (`pending`) and
            the fillers, so by the time the PE reaches this chunk's AV
            matmuls the exp results and the freed AV-PSUM bank are ready.
            Returns this chunk's tail closure for the next chunk to emit."""
            qt = qkT[hp]
            ktt = qkT[4 + hp]
            h0, h1 = 2 * hp, 2 * hp + 1
            qsl = slice(qci * TCH, (qci + 1) * TCH)
            nkt = (TCH // P) * qci + (TCH // P)
            # both heads side by side: cols 0:512 = head h0, 512:1024 = h1
            avp = av_ps.tile([DH + 1, 2 * TCH], F32, tag="avp", name="avp")
            es = {}

            def score_exp(ki):
                ksl = slice(ki * P, (ki + 1) * P)
                diag = ki >= (TCH // P) * qci
                j = ki - (TCH // P) * qci
                off = j * P if diag else 0
                sp = s_ps.tile([P, 2 * TCH], F32, tag="sp", name="sp")
                mm(sp[:, 0:TCH], ktt[0:64, ksl], qt[0:64, qsl], start=True, stop=True)
                mm(
                    sp[:, TCH : 2 * TCH],
                    ktt[64:128, ksl],
                    qt[64:128, qsl],
                    start=True,
                    stop=True,
                )
                if diag:
                    jsl = slice(j * P, (j + 1) * P)
                    jsl2 = slice(TCH + j * P, TCH + (j + 1) * P)
                    mm(
                        sp[:, jsl],
                        ident_sb[:],
                        mtri_sb[:],
                        start=False,
                        stop=True,
                        skip_group_check=True,
                    )
                    mm(
                        sp[:, jsl2],
                        ident_sb[:],
                        mtri_sb[:],
                        start=False,
                        stop=True,
                        skip_group_check=True,
                    )
                # one exp over both heads' live columns
                e = epool.tile([P, 2 * TCH], BF16, tag="e", name="e")
                sp3 = sp[:].rearrange("p (h q) -> p h q", h=2)
                e3 = e[:].rearrange("p (h q) -> p h q", h=2)
                nc.scalar.activation(e3[:, :, off:], sp3[:, :, off:], EXP, scale=SCALE)
                es[ki] = (e, off)

            def av(ki):
                e, off = es.pop(ki)
                mm(
                    avp[:, off:TCH],
                    vbuf[:, ki, h0, :],
                    e[:, off:TCH],
                    start=(ki == 0),
                    stop=(ki == nkt - 1),
                    skip_group_check=True,
                )
                mm(
                    avp[:, TCH + off : 2 * TCH],
                    vbuf[:, ki, h1, :],
                    e[:, TCH + off : 2 * TCH],
                    start=(ki == 0),
                    stop=(ki == nkt - 1),
                    skip_group_check=True,
                )

            score_exp(0)
            score_exp(1)
            if pending is not None:
                pending()
            for f in fillers:
                f()
            av(0)
            av(1)
            for ki in range(2, nkt):
                score_exp(ki)
                av(ki)

            def tail():
                # free avp via two 65-row drains (y rows + denominator row)
                u0 = upool.tile([DH + 1, TCH], F32, tag="u", name="u0")
                u1 = upool.tile([DH + 1, TCH], F32, tag="u", name="u1")
                nc.vector.tensor_copy(u0[:], avp[0 : DH + 1, 0:TCH])
                nc.vector.tensor_copy(u1[:], avp[0 : DH + 1, TCH : 2 * TCH])
                den2 = d2pool.tile([2, TCH], F32R, tag="den2", name="den2")
                nc.sync.dma_start(den2[0:1, :], u0[DH : DH + 1, :].bitcast(F32R))
                nc.sync.dma_start(den2[1:2, :], u1[DH : DH + 1, :].bitcast(F32R))
                # block-diag ones lhsT broadcasts head-0 denom to partitions
                # 0-63 and head-1 to 64-127; rec lives in PSUM so the
                # normalize has a non-SB operand (exempts the equal-base rule)
                rb = aux_ps.tile([P, TCH], F32, tag="aux", name="rb")
                mm(rb[:], ones2_sb[:, :], den2[:], start=True, stop=True)
                rec = aux_ps.tile([P, TCH], F32, tag="aux", name="rec")
                rscr = rpool.tile([P, TCH], F32, tag="rec", name="rscr")
                nc.vector.reciprocal_approx_accurate(rec[:], rb[:], rscr[:])
                nc.vector.tensor_tensor(
                    yT[hp][0:64, qsl], u0[0:DH, :], rec[0:64, :], MULT
                )
                nc.vector.tensor_tensor(
                    yT[hp][64:128, qsl], u1[0:DH, :], rec[64:128, :], MULT
                )

            return tail

        # ---------- emission schedule ----------
        # QK-block(0) feeds C(0); QK-block(h) rides as fillers through C(h-1);
        # the output projection rides through C(3).
        def qk_block_subblocks(hp):
            out = []
            for ft in (hp, 4 + hp):
                for tci in range(n_tch):
                    out.append((ft, tci))
            return out

        for ft, tci in qk_block_subblocks(0):
            qk_subblock(ft, tci)

        from collections import deque

        filler_q = deque(
            [("qk",) + s for h in (1, 2, 3) for s in qk_block_subblocks(h)]
        )
        # per-(hp, qci) filler counts: C0 front-loads 2 subblocks as the
        # V->C bridge; within each C block, finish all fillers by the qci=2
        # tail so the swap-adds complete before the next head pair's scores
        bridge = [filler_q.popleft(), filler_q.popleft()]
        for s in bridge:
            qk_subblock(s[1], s[2])
        counts = {0: [3, 2, 1, 0], 1: [3, 3, 2, 0], 2: [3, 3, 2, 0]}

        pending = None
        for hp in range(n_hp):
            for qci in range(n_tch):
                fillers = []
                if hp < 3:
                    for _ in range(counts[hp][qci]):
                        if filler_q:
                            s = filler_q.popleft()
                            fillers.append(
                                lambda ft=s[1], tci=s[2]: qk_subblock(ft, tci)
                            )
                else:
                    # D rides two chunks behind so its normalize dependency
                    # is long resolved when the PE reaches it
                    if qci >= 2:
                        fillers.append(lambda q=qci - 2: d_group(q))
                pending = c_chunk(hp, qci, fillers, pending)
        pending()
        d_group(2)
        d_group(3)


def _build(S=S_FULL):
    key = ("nc", S)
    if key in _CACHE:
        return _CACHE[key]
    nc = bacc.Bacc("TRN2", target_bir_lowering=False, debug=False, num_devices=8)
    xT = nc.dram_tensor("xT", [P, S // TCH, 8, TCH], BF16, kind="ExternalInput")
    wqkT = nc.dram_tensor("wqkT", [P, 8, FQK], BF16, kind="ExternalInput")
    wvT = nc.dram_tensor("wvT", [P, 8, FV], BF16, kind="ExternalInput")
    woT = nc.dram_tensor("woT", [P, 4, D], BF16, kind="ExternalInput")
    cosF = nc.dram_tensor("cosF", [P, S], F32, kind="ExternalInput")
    sinFpm = nc.dram_tensor("sinFpm", [P, S], F32, kind="ExternalInput")
    mtri = nc.dram_tensor("mtri", [P, P], BF16, kind="ExternalInput")
    ident = nc.dram_tensor("ident", [P, P], BF16, kind="ExternalInput")
    ones2 = nc.dram_tensor("ones2", [2, P], F32R, kind="ExternalInput")
    outp = nc.dram_tensor("outp", [S, D], F16, kind="ExternalOutput")
    with tile.TileContext(nc) as tc:
        _emit(nc, tc, S, xT, wqkT, wvT, woT, cosF, sinFpm, mtri, ident, ones2, outp)
    nc.compile()
    _CACHE[key] = nc
    return nc


def host_inputs(x, wqkv, wo, token_positions, S=S_FULL):
    """Build the 8 per-core input maps (host-side sharding / layout prep)."""
    x = np.asarray(x, dtype=np.float32)
    wqkv = np.asarray(wqkv, dtype=np.float32)
    wo = np.asarray(wo, dtype=np.float32)
    pos = np.asarray(token_positions).astype(np.float32)

    d_model = x.shape[2]
    wq, wk, wv = wqkv[0:d_model], wqkv[d_model : 2 * d_model], wqkv[2 * d_model :]

    inv = np.float32(ROPE_THETA) ** (
        -np.arange(0, DH, 2, dtype=np.float32) / np.float32(DH)
    )  # [32]
    ang = pos[None, :] * inv[:, None]  # [32, S]
    cos32 = np.cos(ang).astype(np.float32)
    sin32 = np.sin(ang).astype(np.float32)
    cosF = np.tile(cos32, (4, 1))  # [128, S]
    sinFpm = np.tile(np.concatenate([sin32, -sin32], axis=0), (2, 1))  # [128, S]

    import ml_dtypes

    a = np.arange(P)
    mtri = np.where(a[:, None] > a[None, :], np.float32(NEG), np.float32(0.0))
    mtri = mtri.astype(ml_dtypes.bfloat16)
    ident = np.eye(P, dtype=ml_dtypes.bfloat16)
    S = x.shape[1]
    ones2 = np.zeros((2, P), np.float32)
    ones2[0, 0:64] = 1.0
    ones2[1, 64:128] = 1.0

    perm64 = np.concatenate([np.arange(0, DH, 2), np.arange(1, DH, 2)])

    in_maps = []
    for ci in range(8):
        bi, hg = divmod(ci, 2)
        xT = x[bi].T  # [d, s]
        rows = []
        for blk in (wq, wk):
            for h in range(hg * NH_CORE, (hg + 1) * NH_CORE):
                rows.append(blk[h * DH : (h + 1) * DH][perm64])
        wqkT = np.concatenate(rows, axis=0).T  # [d, fqk]
        wvT = wv[hg * FV : (hg + 1) * FV].T  # [d, fv]
        woT = wo[:, hg * FV : (hg + 1) * FV].T  # [fv, d]
        # partition-major device layouts: one contiguous run per partition
        xT = np.ascontiguousarray(
            xT.reshape(8, P, S // TCH, TCH).transpose(1, 2, 0, 3)
        ).astype(ml_dtypes.bfloat16)  # [p, tch, eo, t]
        wqkT = np.ascontiguousarray(
            wqkT.reshape(8, P, FQK).transpose(1, 0, 2)
        ).astype(ml_dtypes.bfloat16)  # [p, eo, f]
        wvT = np.ascontiguousarray(
            wvT.reshape(8, P, FV).transpose(1, 0, 2)
        ).astype(ml_dtypes.bfloat16)  # [p, eo, f]
        woT = np.ascontiguousarray(
            woT.reshape(4, P, D).transpose(1, 0, 2)
        ).astype(ml_dtypes.bfloat16)  # [p, co, j]
        in_maps.append(
            {
                "xT": xT,
                "wqkT": wqkT,
                "wvT": wvT,
                "woT": woT,
                "cosF": cosF,
                "sinFpm": sinFpm,
                "mtri": mtri,
                "ident": ident,
                "ones2": ones2,
            }
        )
    return in_maps


def _install_ntff_hook():
    """Recreate the antenv.axon_hooks NTFF profile hook this image lacks
    (same ctypes shim trn_agent_boot would register). Dev/profiling only."""
    import contextlib
    import ctypes
    import os
    import types

    try:
        import antenv.axon_hooks  # noqa: F401

        return
    except ImportError:
        pass
    so_path = "/opt/axon/libaxon_pjrt.so"
    if not os.path.exists(so_path):
        return
    lib = ctypes.CDLL(so_path)
    if not hasattr(lib, "axon_start_nrt_profile"):
        return
    lib.axon_start_nrt_profile.argtypes = [
        ctypes.POINTER(ctypes.c_int64),
        ctypes.c_size_t,
    ]
    lib.axon_start_nrt_profile.restype = ctypes.c_int64
    lib.axon_stop_nrt_profile.argtypes = [ctypes.c_char_p]
    lib.axon_stop_nrt_profile.restype = ctypes.c_int64

    @contextlib.contextmanager
    def _hook(output_dir, device_ids):
        import jax

        jax.devices()
        if device_ids:
            ids = (ctypes.c_int64 * len(device_ids))(*device_ids)
            rc = lib.axon_start_nrt_profile(ids, len(device_ids))
        else:
            rc = lib.axon_start_nrt_profile(None, 0)
        if rc != 0:
            raise RuntimeError(f"axon_start_nrt_profile rc={rc}")
        try:
            yield
        finally:
            n = lib.axon_stop_nrt_profile(str(output_dir).encode())
            if n < 0:
                raise RuntimeError(f"axon_stop_nrt_profile rc={n}")

    import antenv
    from concourse import bass_utils as _bu

    _bu.upload_artifacts = lambda d: d  # no bucket access in this container
    mod = types.ModuleType("antenv.axon_hooks")
    mod.get_axon_ntff_profile_hook = lambda: _hook
    mod.set_axon_ntff_profile_hook = lambda h: None
    sys.modules["antenv.axon_hooks"] = mod
    antenv.axon_hooks = mod


def kernel(x, wqkv, wo, token_positions, trace=False):
    if trace:
        _install_ntff_hook()
    nc = _build()
    in_maps = host_inputs(x, wqkv, wo, token_positions)
    res = run_bass_kernel_spmd(nc, in_maps, core_ids=list(range(8)), trace=trace)
    parts = [res.results[ci]["outp"].astype(np.float32) for ci in range(8)]
    out = np.stack([parts[2 * bi] + parts[2 * bi + 1] for bi in range(B)], axis=0)
    if trace:
        kernel.last_result = res
    return out


# revision 17
# speedup vs baseline: 1.2120x; 1.2120x over previous
"""Causal multi-head self-attention (b=4, s=2048, d_model=1024, 16 heads) on 8
Trainium2 NeuronCores.

Sharding: core c handles batch c//2 and head-group c%2 (8 of 16 heads):
  - wqkv row-split by head (tensor parallel), wo column-split by head.
  - Each core returns the partial output projection [s, d_model] for its head
    group; the host sums the two partials of each batch while unsharding (the
    pairwise all-reduce of the TP split).

v2 restructure (from 465us baseline) driven by trace analysis:
  - The baseline lost ~150us to HAM clock-gate oscillation (PE idle gaps of
    2-10us at phase seams / chunk tails re-throttle the PE to 1.2 GHz) and
    ~20us to a descriptor-flooded startup (a [128,16,8,1] ones DMA emitted
    ~16k 2-byte descriptors before the first weight load).
  - All of x (4MB) + all weights now live in SBUF, loaded once (the baseline
    re-streamed x per projection phase: 20MB, starving the PE).
  - The ones column of V comes from one strided memset, not a DMA.
  - Emission order keeps the PE continuously fed: V proj, QK-block(0), then
    attention C(0..3) with the remaining QK projection emitted as ~1.7us
    fillers at every chunk tail (where the softmax-denominator chain used to
    stall the PE), and the output projection D interleaved into C(3)'s tails.
  - The AV PSUM tile is freed early: unnormalized y rows drain to SBUF (DVE)
    and denominators to f32r staging (Scalar) right after the last AV matmul;
    the normalize multiply runs later from SBUF off the critical path.
  - Output partials ship as f16 (host sums in f32).

Per-core dataflow (heavy matmuls bf16, fp32 PSUM; softmax denominator f32r):
  V) v[t, f] token-major, stored [t, ktile, head, 65] with a ones column per
     head -- the AV matmul then yields softmax denominators for free.
  QK) qkT[f, t] feature-major per 128-row f-tile with RoPE fused:
      qk' = cos * qk + DMA-swap-add( sin_pm * qk ), sin_pm = [+sin | -sin],
      SWDGE accum_op=add swaps the 32-row halves of each 64-row head block.
  C) per (head pair, q-chunk of 512) over causal k-tiles of 128:
      scores^T two K=64 matmuls (heads at partition halves); causal mask via
      identity @ mtri accumulated onto the diagonal PSUM block; exp on ScalarE
      (scale=1/8 folded); AV with lhsT = [V_h | 1] -> PSUM [65, q], row 64 =
      denominator; drain PSUM early; ones2 K=2 matmul broadcasts denominators
      across partitions; DVE approx-reciprocal; normalize SBUF->SBUF.
  D) output projection vs wo columns, f16 partial DMA'd out; host sums core
     pairs while unsharding.
"""

import sys

if "/opt/trn_rl_repo" not in sys.path:
    sys.path.insert(0, "/opt/trn_rl_repo")

from contextlib import ExitStack

import numpy as np

import concourse.bass as bass  # noqa: F401
import concourse.tile as tile
from concourse import bacc, mybir
from concourse.bass_utils import run_bass_kernel_spmd

F32 = mybir.dt.float32
F32R = mybir.dt.float32r
F16 = mybir.dt.float16
BF16 = mybir.dt.bfloat16
EXP = mybir.ActivationFunctionType.Exp
MULT = mybir.AluOpType.mult
ADD = mybir.AluOpType.add

# Problem constants
B, S_FULL, D = 4, 2048, 1024
NH_CORE = 8      # heads per core
DH = 64          # head dim
FQK = 1024       # Q+K features per core
FV = 512         # V features per core
P = 128
TCH = 512        # q/t chunk size
NEG = -1.0e30
ROPE_THETA = 10000.0
SCALE = 1.0 / 8.0  # 1/sqrt(DH)

_CACHE = {}


def _emit(nc, tc, S, xT, wqkT, wvT, woT, cosF, sinFpm, mtri, ident, ones2, outp):
    n_tch = S // TCH          # 4
    n_kt = S // P             # 16
    n_hp = NH_CORE // 2       # 4
    mm = nc.tensor.matmul

    with ExitStack() as ctx:
        # ---------- persistent buffers ----------
        persist = ctx.enter_context(tc.tile_pool(name="persist", bufs=1))
        xsb = [
            persist.tile([P, 8, TCH], BF16, tag=f"xsb{t}", name=f"xsb{t}")
            for t in range(n_tch)
        ]
        wqk_sb = persist.tile([P, 8, FQK], BF16, tag="wqk")
        wv_sb = persist.tile([P, 8, FV], BF16, tag="wv")
        wo_sb = persist.tile([P, 4, D], BF16, tag="wo")
        cos_sb = persist.tile([P, S], F32, tag="cos")
        sin_sb = persist.tile([P, S], F32, tag="sin")
        qkT = [
            persist.tile([P, S], BF16, tag=f"qkT{ft}", name=f"qkT{ft}")
            for ft in range(8)
        ]
        vbuf = persist.tile([P, n_kt, NH_CORE, DH + 1], BF16, tag="vbuf")
        yT = [
            persist.tile([P, S], BF16, tag=f"yT{hp}", name=f"yT{hp}")
            for hp in range(n_hp)
        ]
        ident_sb = persist.tile([P, P], BF16, tag="ident")
        mtri_sb = persist.tile([P, P], BF16, tag="mtri")
        ones2_sb = persist.tile([2, P], F32R, tag="ones2")
        scr = persist.tile([1, P], BF16, tag="scr")

        # input DMAs, most-urgent first (wv + x feed the V projection); all
        # host arrays are pre-rearranged partition-major so each load is one
        # contiguous descriptor per partition
        nc.sync.dma_start(wv_sb[:, 0:4, :], wvT.ap()[:, 0:4, :])
        nc.sync.dma_start(wv_sb[:, 4:8, :], wvT.ap()[:, 4:8, :])
        nc.sync.dma_start(xsb[0][:, 0:4, :], xT.ap()[:, 0, 0:4, :])
        nc.sync.dma_start(xsb[0][:, 4:8, :], xT.ap()[:, 0, 4:8, :])
        for t in range(1, n_tch):
            nc.sync.dma_start(xsb[t][:], xT.ap()[:, t, :, :])
        nc.sync.dma_start(wqk_sb[:], wqkT.ap()[:, :, :])
        nc.sync.dma_start(cos_sb[:], cosF.ap()[:, :])
        nc.sync.dma_start(sin_sb[:], sinFpm.ap()[:, :])
        nc.sync.dma_start(ident_sb[:], ident.ap()[:, :])
        nc.sync.dma_start(mtri_sb[:], mtri.ap()[:, :])
        nc.sync.dma_start(ones2_sb[:], ones2.ap()[:, :])
        nc.sync.dma_start(wo_sb[:], woT.ap()[:, :, :])
        # ones column of V (softmax denominator trick) -- memset, not DMA
        nc.vector.memset(vbuf[:, :, :, DH : DH + 1], 1.0)
        # preload the ScalarE exp table before C needs it
        nc.vector.memset(scr[:], 0.0)
        nc.scalar.activation(scr[:], scr[:], EXP, scale=SCALE)

        # ---------- V projection (token-major) ----------
        with ExitStack() as vb:
            v_ps = vb.enter_context(tc.tile_pool(name="v_psum", bufs=2, space="PSUM"))
            for kt in range(n_kt):
                tci, tti = divmod(kt, n_tch)
                vps = v_ps.tile([P, FV], F32, tag="vps", name="vps")
                for ec in range(8):
                    mm(
                        vps[:],
                        xsb[tci][:, ec, tti * P : (tti + 1) * P],
                        wv_sb[:, ec, :],
                        start=(ec == 0),
                        stop=(ec == 7),
                    )
                nc.scalar.copy(vbuf[:, kt, :, 0:DH], vps[:])

        # ---------- C-epoch pools (PSUM: sp 4 banks, avp 2, aux 2) ----------
        apool = ctx.enter_context(tc.tile_pool(name="btmp", bufs=2))
        epool = ctx.enter_context(tc.tile_pool(name="expS", bufs=5))
        d2pool = ctx.enter_context(tc.tile_pool(name="den2", bufs=2))
        upool = ctx.enter_context(tc.tile_pool(name="ustage", bufs=4))
        rpool = ctx.enter_context(tc.tile_pool(name="recb", bufs=2))
        opool = ctx.enter_context(tc.tile_pool(name="outsb", bufs=3))
        s_ps = ctx.enter_context(tc.tile_pool(name="s_psum", bufs=2, space="PSUM"))
        av_ps = ctx.enter_context(tc.tile_pool(name="av_psum", bufs=1, space="PSUM"))
        aux_ps = ctx.enter_context(tc.tile_pool(name="aux_psum", bufs=2, space="PSUM"))

        btbuf = {}

        def qk_subblock(ft, tci):
            # one f-tile x one token chunk of the QK projection, RoPE fused
            tsl = slice(tci * TCH, (tci + 1) * TCH)
            if tci == 0:
                btbuf[ft] = apool.tile([P, S], BF16, tag="bt", name=f"bt{ft}")
            btf = btbuf[ft]
            ps = aux_ps.tile([P, TCH], F32, tag="aux", name="qkps")
            for ec in range(8):
                mm(
                    ps[:],
                    wqk_sb[:, ec, ft * P : (ft + 1) * P],
                    xsb[tci][:, ec, :],
                    start=(ec == 0),
                    stop=(ec == 7),
                )
            nc.vector.tensor_tensor(qkT[ft][:, tsl], ps[:], cos_sb[:, tsl], MULT)
            nc.vector.tensor_tensor(btf[:, tsl], ps[:], sin_sb[:, tsl], MULT)
            if tci == n_tch - 1:
                # rotate-half: swap-add the 32-row halves of each 64-row block
                for blk in range(4):
                    a = blk * 32
                    c2 = a ^ 32
                    nc.gpsimd.dma_start(
                        qkT[ft][c2 : c2 + 32, :], btf[a : a + 32, :], accum_op=ADD
                    )

        def d_group(qci):
            # output projection for the 4 token tiles of chunk qci
            for tti in range(qci * n_tch, qci * n_tch + 4):
                tsl = slice(tti * P, (tti + 1) * P)
                for jc in range(2):
                    jsl = slice(jc * TCH, (jc + 1) * TCH)
                    op = aux_ps.tile([P, TCH], F32, tag="aux", name="dps")
                    for cc in range(4):
                        mm(
                            op[:],
                            yT[cc][:, tsl],
                            wo_sb[:, cc, jsl],
                            start=(cc == 0),
                            stop=(cc == 3),
                        )
                    ot = opool.tile([P, TCH], F16, tag="ot", name="ot")
                    if jc == 0:
                        nc.scalar.copy(ot[:], op[:])
                    else:
                        nc.vector.tensor_copy(ot[:], op[:])
                    nc.sync.dma_start(outp.ap()[tsl, jsl], ot[:])

        def c_chunk(hp, qci, fillers):
            qt = qkT[hp]
            ktt = qkT[4 + hp]
            h0, h1 = 2 * hp, 2 * hp + 1
            qsl = slice(qci * TCH, (qci + 1) * TCH)
            nkt = (TCH // P) * qci + (TCH // P)
            # both heads side by side: cols 0:512 = head h0, 512:1024 = h1
            avp = av_ps.tile([DH + 1, 2 * TCH], F32, tag="avp", name="avp")
            es = {}

            def score_exp(ki):
                ksl = slice(ki * P, (ki + 1) * P)
                diag = ki >= (TCH // P) * qci
                j = ki - (TCH // P) * qci
                off = j * P if diag else 0
                sp = s_ps.tile([P, 2 * TCH], F32, tag="sp", name="sp")
                mm(sp[:, 0:TCH], ktt[0:64, ksl], qt[0:64, qsl], start=True, stop=True)
                mm(
                    sp[:, TCH : 2 * TCH],
                    ktt[64:128, ksl],
                    qt[64:128, qsl],
                    start=True,
                    stop=True,
                )
                if diag:
                    jsl = slice(j * P, (j + 1) * P)
                    jsl2 = slice(TCH + j * P, TCH + (j + 1) * P)
                    mm(
                        sp[:, jsl],
                        ident_sb[:],
                        mtri_sb[:],
                        start=False,
                        stop=True,
                        skip_group_check=True,
                    )
                    mm(
                        sp[:, jsl2],
                        ident_sb[:],
                        mtri_sb[:],
                        start=False,
                        stop=True,
                        skip_group_check=True,
                    )
                # one exp over both heads' live columns
                e = epool.tile([P, 2 * TCH], BF16, tag="e", name="e")
                sp3 = sp[:].rearrange("p (h q) -> p h q", h=2)
                e3 = e[:].rearrange("p (h q) -> p h q", h=2)
                nc.scalar.activation(e3[:, :, off:], sp3[:, :, off:], EXP, scale=SCALE)
                es[ki] = (e, off)

            def av(ki):
                e, off = es.pop(ki)
                mm(
                    avp[:, off:TCH],
                    vbuf[:, ki, h0, :],
                    e[:, off:TCH],
                    start=(ki == 0),
                    stop=(ki == nkt - 1),
                    skip_group_check=True,
                )
                mm(
                    avp[:, TCH + off : 2 * TCH],
                    vbuf[:, ki, h1, :],
                    e[:, TCH + off : 2 * TCH],
                    start=(ki == 0),
                    stop=(ki == nkt - 1),
                    skip_group_check=True,
                )

            for ki in range(nkt):
                score_exp(ki)
                av(ki)
            # --- chunk tail: free avp via two 65-row drains (y rows + the
            # denominator row together); ScalarE stays a pure exp stream
            u0 = upool.tile([DH + 1, TCH], F32, tag="u", name="u0")
            u1 = upool.tile([DH + 1, TCH], F32, tag="u", name="u1")
            nc.vector.tensor_copy(u0[:], avp[0 : DH + 1, 0:TCH])
            nc.vector.tensor_copy(u1[:], avp[0 : DH + 1, TCH : 2 * TCH])
            for f in fillers:
                f()
            den2 = d2pool.tile([2, TCH], F32R, tag="den2", name="den2")
            nc.sync.dma_start(den2[0:1, :], u0[DH : DH + 1, :].bitcast(F32R))
            nc.sync.dma_start(den2[1:2, :], u1[DH : DH + 1, :].bitcast(F32R))
            # block-diag ones lhsT broadcasts head-0 denom to partitions 0-63
            # and head-1 to 64-127; rec lives in PSUM so the normalize has a
            # non-SB operand (exempts the equal-base-partition rule)
            rb = aux_ps.tile([P, TCH], F32, tag="aux", name="rb")
            mm(rb[:], ones2_sb[:, :], den2[:], start=True, stop=True)
            rec = aux_ps.tile([P, TCH], F32, tag="aux", name="rec")
            rscr = rpool.tile([P, TCH], F32, tag="rec", name="rscr")
            nc.vector.reciprocal_approx_accurate(rec[:], rb[:], rscr[:])
            nc.vector.tensor_tensor(yT[hp][0:64, qsl], u0[0:DH, :], rec[0:64, :], MULT)
            nc.vector.tensor_tensor(
                yT[hp][64:128, qsl], u1[0:DH, :], rec[64:128, :], MULT
            )

        # ---------- emission schedule ----------
        # QK-block(0) feeds C(0); QK-block(h) rides as fillers through C(h-1);
        # the output projection rides through C(3).
        def qk_block_subblocks(hp):
            out = []
            for ft in (hp, 4 + hp):
                for tci in range(n_tch):
                    out.append((ft, tci))
            return out

        for ft, tci in qk_block_subblocks(0):
            qk_subblock(ft, tci)

        from collections import deque

        filler_q = deque(
            [("qk",) + s for h in (1, 2, 3) for s in qk_block_subblocks(h)]
        )
        # per-(hp, qci) filler counts: C0 front-loads 2 subblocks as the
        # V->C bridge; within each C block, finish all fillers by the qci=2
        # tail so the swap-adds complete before the next head pair's scores
        bridge = [filler_q.popleft(), filler_q.popleft()]
        for s in bridge:
            qk_subblock(s[1], s[2])
        counts = {0: [3, 2, 1, 0], 1: [3, 3, 2, 0], 2: [3, 3, 2, 0]}

        for hp in range(n_hp):
            for qci in range(n_tch):
                fillers = []
                if hp < 3:
                    for _ in range(counts[hp][qci]):
                        if filler_q:
                            s = filler_q.popleft()
                            fillers.append(
                                lambda ft=s[1], tci=s[2]: qk_subblock(ft, tci)
                            )
                else:
                    if qci >= 1:
                        fillers.append(lambda q=qci - 1: d_group(q))
                c_chunk(hp, qci, fillers)
        d_group(3)


def _build(S=S_FULL):
    key = ("nc", S)
    if key in _CACHE:
        return _CACHE[key]
    nc = bacc.Bacc("TRN2", target_bir_lowering=False, debug=False, num_devices=8)
    xT = nc.dram_tensor("xT", [P, S // TCH, 8, TCH], BF16, kind="ExternalInput")
    wqkT = nc.dram_tensor("wqkT", [P, 8, FQK], BF16, kind="ExternalInput")
    wvT = nc.dram_tensor("wvT", [P, 8, FV], BF16, kind="ExternalInput")
    woT = nc.dram_tensor("woT", [P, 4, D], BF16, kind="ExternalInput")
    cosF = nc.dram_tensor("cosF", [P, S], F32, kind="ExternalInput")
    sinFpm = nc.dram_tensor("sinFpm", [P, S], F32, kind="ExternalInput")
    mtri = nc.dram_tensor("mtri", [P, P], BF16, kind="ExternalInput")
    ident = nc.dram_tensor("ident", [P, P], BF16, kind="ExternalInput")
    ones2 = nc.dram_tensor("ones2", [2, P], F32R, kind="ExternalInput")
    outp = nc.dram_tensor("outp", [S, D], F16, kind="ExternalOutput")
    with tile.TileContext(nc) as tc:
        _emit(nc, tc, S, xT, wqkT, wvT, woT, cosF, sinFpm, mtri, ident, ones2, outp)
    nc.compile()
    _CACHE[key] = nc
    return nc


def host_inputs(x, wqkv, wo, token_positions, S=S_FULL):
    """Build the 8 per-core input maps (host-side sharding / layout prep)."""
    x = np.asarray(x, dtype=np.float32)
    wqkv = np.asarray(wqkv, dtype=np.float32)
    wo = np.asarray(wo, dtype=np.float32)
    pos = np.asarray(token_positions).astype(np.float32)

    d_model = x.shape[2]
    wq, wk, wv = wqkv[0:d_model], wqkv[d_model : 2 * d_model], wqkv[2 * d_model :]

    inv = np.float32(ROPE_THETA) ** (
        -np.arange(0, DH, 2, dtype=np.float32) / np.float32(DH)
    )  # [32]
    ang = pos[None, :] * inv[:, None]  # [32, S]
    cos32 = np.cos(ang).astype(np.float32)
    sin32 = np.sin(ang).astype(np.float32)
    cosF = np.tile(cos32, (4, 1))  # [128, S]
    sinFpm = np.tile(np.concatenate([sin32, -sin32], axis=0), (2, 1))  # [128, S]

    import ml_dtypes

    a = np.arange(P)
    mtri = np.where(a[:, None] > a[None, :], np.float32(NEG), np.float32(0.0))
    mtri = mtri.astype(ml_dtypes.bfloat16)
    ident = np.eye(P, dtype=ml_dtypes.bfloat16)
    S = x.shape[1]
    ones2 = np.zeros((2, P), np.float32)
    ones2[0, 0:64] = 1.0
    ones2[1, 64:128] = 1.0

    perm64 = np.concatenate([np.arange(0, DH, 2), np.arange(1, DH, 2)])

    in_maps = []
    for ci in range(8):
        bi, hg = divmod(ci, 2)
        xT = x[bi].T  # [d, s]
        rows = []
        for blk in (wq, wk):
            for h in range(hg * NH_CORE, (hg + 1) * NH_CORE):
                rows.append(blk[h * DH : (h + 1) * DH][perm64])
        wqkT = np.concatenate(rows, axis=0).T  # [d, fqk]
        wvT = wv[hg * FV : (hg + 1) * FV].T  # [d, fv]
        woT = wo[:, hg * FV : (hg + 1) * FV].T  # [fv, d]
        # partition-major device layouts: one contiguous run per partition
        xT = np.ascontiguousarray(
            xT.reshape(8, P, S // TCH, TCH).transpose(1, 2, 0, 3)
        ).astype(ml_dtypes.bfloat16)  # [p, tch, eo, t]
        wqkT = np.ascontiguousarray(
            wqkT.reshape(8, P, FQK).transpose(1, 0, 2)
        ).astype(ml_dtypes.bfloat16)  # [p, eo, f]
        wvT = np.ascontiguousarray(
            wvT.reshape(8, P, FV).transpose(1, 0, 2)
        ).astype(ml_dtypes.bfloat16)  # [p, eo, f]
        woT = np.ascontiguousarray(
            woT.reshape(4, P, D).transpose(1, 0, 2)
        ).astype(ml_dtypes.bfloat16)  # [p, co, j]
        in_maps.append(
            {
                "xT": xT,
                "wqkT": wqkT,
                "wvT": wvT,
                "woT": woT,
                "cosF": cosF,
                "sinFpm": sinFpm,
                "mtri": mtri,
                "ident": ident,
                "ones2": ones2,
            }
        )
    return in_maps


def _install_ntff_hook():
    """Recreate the antenv.axon_hooks NTFF profile hook this image lacks
    (same ctypes shim trn_agent_boot would register). Dev/profiling only."""
    import contextlib
    import ctypes
    import os
    import types

    try:
        import antenv.axon_hooks  # noqa: F401

        return
    except ImportError:
        pass
    so_path = "/opt/axon/libaxon_pjrt.so"
    if not os.path.exists(so_path):
        return
    lib = ctypes.CDLL(so_path)
    if not hasattr(lib, "axon_start_nrt_profile"):
        return
    lib.axon_start_nrt_profile.argtypes = [
        ctypes.POINTER(ctypes.c_int64),
        ctypes.c_size_t,
    ]
    lib.axon_start_nrt_profile.restype = ctypes.c_int64
    lib.axon_stop_nrt_profile.argtypes = [ctypes.c_char_p]
    lib.axon_stop_nrt_profile.restype = ctypes.c_int64

    @contextlib.contextmanager
    def _hook(output_dir, device_ids):
        import jax

        jax.devices()
        if device_ids:
            ids = (ctypes.c_int64 * len(device_ids))(*device_ids)
            rc = lib.axon_start_nrt_profile(ids, len(device_ids))
        else:
            rc = lib.axon_start_nrt_profile(None, 0)
        if rc != 0:
            raise RuntimeError(f"axon_start_nrt_profile rc={rc}")
        try:
            yield
        finally:
            n = lib.axon_stop_nrt_profile(str(output_dir).encode())
            if n < 0:
                raise RuntimeError(f"axon_stop_nrt_profile rc={n}")

    import antenv
    from concourse import bass_utils as _bu

    _bu.upload_artifacts = lambda d: d  # no bucket access in this container
    mod = types.ModuleType("antenv.axon_hooks")
    mod.get_axon_ntff_profile_hook = lambda: _hook
    mod.set_axon_ntff_profile_hook = lambda h: None
    sys.modules["antenv.axon_hooks"] = mod
    antenv.axon_hooks = mod


def kernel(x, wqkv, wo, token_positions, trace=False):
    if trace:
        _install_ntff_hook()
    nc = _build()
    in_maps = host_inputs(x, wqkv, wo, token_positions)
    res = run_bass_kernel_spmd(nc, in_maps, core_ids=list(range(8)), trace=trace)
    parts = [res.results[ci]["outp"].astype(np.float32) for ci in range(8)]
    out = np.stack([parts[2 * bi] + parts[2 * bi + 1] for bi in range(B)], axis=0)
    if trace:
        kernel.last_result = res
    return out
